# revision 41
# baseline (speedup 1.0000x reference)
"""Causal self-attention (non-masked softmax) for TRN2, 8 NeuronCores.

Sharding: 2-way data parallel over batch x 4-way tensor parallel over heads.
Core c: batch c//4, head group c%4 (4 heads, 256 features). Host sums the
4 row-parallel c_proj partials per batch (fp16 partials) and adds the
host-folded biases (b_proj + b_attn_v @ W_proj; v-bias commutes through
softmax since sum_k p_k = 1).

Engine split per core:
  PE:  QKV projections, S^T (row-packed head pairs), PV (v augmented with a
       ones column so psum row 64 is the softmax denominator), c_proj.
  ACT: exp over S^T psum tiles [128,1024] (~1.15us each; with the PE's
       ~137us of matmuls at 2.4GHz the two streams are nearly balanced),
       plus v/qk staging copies in the PE-bound lead-in phase.
  DVE: one exp tile per ACT-bound group via two custom ops (compound base
       (1+s/256+s^2/(2*256^2)) then 8 chained squarings; ~6e-3 rel err at
       bf16 out), qk bias staging, denominator reciprocal, y normalize
       muls, c_proj psum->fp16 staging.
  GPSIMD: partition_broadcast of 1/denom rows, input DMA issue.

PSUM: tag s = 2x[128,1024] (S tiles, double buffered), pvd = 2x[128,512]
(PV accumulators for the in-flight head pair), fill = 2x[128,512]
(projection / c_proj scratch) = 8 banks total.

Schedule: 8 attention groups (j, pair) ordered all-pair0-then-all-pair1 so
the pair-1 q/k projections spread as fillers over the pair-0 groups and
cproj(j) spreads over the pair-1 groups; every group carries ~3.5us of
filler PE work against its 18.4us exp stream. Steady state per step: the
S pipeline runs two tiles ahead of the exp stream (S(i+2) emitted at step
i; S(0)/S(1) of the next group pre-emitted during the last two steps) and
PV(i) is deferred one step (three for DVE-exp steps and i=0) so the
in-order PE never blocks on exp completion or on the previous group's
pvd-freeing copies. Fillers are split into <=1.7us pieces and paced into
the back half of ACT-bound groups where the PE has accumulated slack.
Junk matmuls bridge the DMA-bound lead-in and the normalize tail so the
PE HAM clock gate stays at 2.4GHz for the whole kernel.
"""

import numpy as np

B, T, H, NH, HD = 2, 2048, 1024, 16, 64
P, FG = 128, 256
NQ = 512          # query block per j
NJ = 4            # T/NQ
NI = 16           # key chunks of 128
KH = 8            # hidden chunks of 128
W3 = 3 * FG       # wqkv row width per core (768)
NCORES = 8

# i-steps per (j,pair) group whose exp runs on DVE instead of ACT
# (only in groups where the ACT exp stream, not the PE, is the limiter)
DVE_EXP_I = {
    (1, 0): (5, 10),
    (2, 0): (5, 10),
    (3, 0): (5, 10),
    (0, 1): (5,),
    (1, 1): (5,),
    (2, 1): (5,),
    (3, 1): (5,),
}

_CACHE = {}


def _register_dve_exp():
    """Register the custom DVE exp ops (idempotent). exp(s) ~= base^(256)
    with base = 1 + s/256 + (s/256)^2/2: op1 computes the base, op2 squares
    eight times. Validated on hw: max rel err ~6e-3 over s in [-8, 8]."""
    import concourse.dve_ops as dve_ops
    from concourse.dve_ops import DveOp, OPS, CUSTOM_DVE_SPECS, _SUB_OPCODE_FOR_NAME
    from concourse.dve_spec import Spec, Src0, C0, C1, C2, lower, sq
    from concourse.dve_uop import DveOpSpec

    have = {o.name: o for o in OPS}
    if "EXP_BASE_XK" in have:
        return have["EXP_BASE_XK"], have["EXP_SQ8_XK"]

    def make(name, body, ref):
        spec = Spec(body=body, reference=ref)
        row = max(_SUB_OPCODE_FOR_NAME.values()) + 1
        _SUB_OPCODE_FOR_NAME[name] = row
        shas = {}
        for ver in ("v3",):
            tmp = DveOpSpec(name=name, opcode=row, uops=lower(spec, ver=ver),
                            rd1_en=False)
            shas[ver] = tmp.sha(ver)
        op = DveOp(name, spec, subdim=False, uops_sha=shas)
        OPS.append(op)
        CUSTOM_DVE_SPECS[name] = spec
        return op

    w = Src0 * C0
    base_body = C1 + w * (C1 + w * C2)

    def ref_base(in0, in1, s0, s1, imm2):
        ww = in0.astype(np.float32) * np.float32(s0)
        return (np.float32(s1) + ww * (np.float32(s1) + ww * np.float32(imm2))
                ).astype(np.float32)

    x = Src0
    for _ in range(8):
        x = sq(x)

    def ref_sq8(in0, in1, s0, s1, imm2):
        v = in0.astype(np.float32)
        for _ in range(8):
            v = (v * v).astype(np.float32)
        return v

    return make("EXP_BASE_XK", base_body, ref_base), make("EXP_SQ8_XK", x, ref_sq8)


def _build():
    import concourse.bacc as bacc
    import concourse.mybir as mybir
    import concourse.tile as tile

    EXP_BASE, EXP_SQ8 = _register_dve_exp()

    fp32 = mybir.dt.float32
    fp16 = mybir.dt.float16
    bf16 = mybir.dt.bfloat16

    nc = bacc.Bacc("TRN2", debug=False)
    xT = nc.dram_tensor("xT", [H, T], bf16, kind="ExternalInput").ap()
    wqkv = nc.dram_tensor("wqkv", [H, W3], bf16, kind="ExternalInput").ap()
    bqk = nc.dram_tensor("bqk", [2 * FG], fp32, kind="ExternalInput").ap()
    wp = nc.dram_tensor("wp", [FG, H], bf16, kind="ExternalInput").ap()
    out = nc.dram_tensor("out", [T, H], fp16, kind="ExternalOutput").ap()

    with tile.TileContext(nc) as tc:
        _emit(nc, tc, mybir, EXP_BASE, EXP_SQ8, xT, wqkv, bqk, wp, out)
    nc.compile()
    return nc


def _emit(nc, tc, mybir, EXP_BASE, EXP_SQ8, xT, wqkv, bqk, wp, out):
    from contextlib import ExitStack

    fp32 = mybir.dt.float32
    fp16 = mybir.dt.float16
    bf16 = mybir.dt.bfloat16
    Exp = mybir.ActivationFunctionType.Exp
    Ident = mybir.ActivationFunctionType.Identity

    with ExitStack() as ctx:
        pool = lambda name, bufs=1, space="SBUF": ctx.enter_context(
            tc.tile_pool(name=name, bufs=bufs, space=space)
        )

        # ---- persistent SBUF tiles (fine-grained so deps stay fine) ----
        const = pool("const")
        bias0 = const.tile([P, 1], fp32)
        nc.vector.memset(bias0[:], 0.0)
        junk = const.tile([P, NQ], bf16)        # PE warmup operand
        nc.vector.memset(junk[:], 0.001)
        warm = const.tile([P, 8], fp32)
        nc.vector.memset(warm[:], 0.0)
        bqk_sb = const.tile([P, 4], fp32)       # bias col per m-chunk
        nc.sync.dma_start(bqk_sb[:], bqk.rearrange("(m p) -> p m", p=P))

        xtp = pool("xt")
        # per (hidden chunk, token half) so early readers don't wait all DMA
        xts = [[xtp.tile([P, T // 2], bf16, name=f"xt{k}_{h}") for h in range(2)] for k in range(KH)]
        wqp = pool("wq")
        # q/k and v weight columns in separate tiles so the critical q/k
        # prefix DMA isn't diluted by v columns
        wqks = [wqp.tile([P, 2 * FG], bf16, name=f"wqk{k}") for k in range(KH)]
        wvs = [wqp.tile([P, FG], bf16, name=f"wv{k}") for k in range(KH)]
        wpp = pool("wpp")
        wps = [wpp.tile([P, H], bf16, name=f"wp{k}") for k in range(2)]
        qkp = pool("qk")
        qks = [[qkp.tile([P, NQ], bf16, name=f"qk{m}_{j}") for j in range(NJ)] for m in range(4)]
        vp = pool("v")
        vts = [vp.tile([P, 4 * P], bf16, name=f"v{t}") for t in range(NI)]
        yp = pool("y")
        ys = [[yp.tile([P, NQ], bf16, name=f"y{k}_{j}") for j in range(NJ)] for k in range(2)]

        # ---- working pools ----
        epool = pool("e", bufs=4)
        scrp = pool("scr", bufs=2)      # DVE exp fp32 intermediate
        pvsp = pool("pvs", bufs=4)      # staged PV numerators
        rcpp = pool("rcp", bufs=4)      # 1/denom rows + broadcasts
        outp = pool("o", bufs=4)        # fp16 out staging
        ps = ctx.enter_context(tc.tile_pool(name="ps", bufs=2, space="PSUM"))

        # ---- warmups (off critical path): ACT exp table, PE p-state ----
        nc.scalar.activation(warm[:], warm[:], Exp, bias=bias0[:, 0:1])
        for t in range(NI):  # mark ones col + zero pad of v tiles
            v4 = vts[t].rearrange("p (h c) -> p h c", h=4, c=P)
            nc.vector.memset(v4[:, :, 64:66], 0.0)
            nc.vector.memset(v4[:, :, 64:65], 1.0)

        # ---- input DMAs, spread across idle engine queues (not ACT:
        # scalar-queue DMA issues would steal ~6us from the exp stream) ----
        dq = [nc.sync, nc.gpsimd]
        di = [0]

        def dma(dst, src):
            dq[di[0] % 2].dma_start(dst, src)
            di[0] += 1

        for k in range(KH):  # critical prefix: q/k weights + first token half
            dma(wqks[k][:], wqkv[k * P : (k + 1) * P, 0 : 2 * FG])
            dma(xts[k][0][:], xT[k * P : (k + 1) * P, 0 : 2 * NQ])
        for k in range(KH):  # v weights (needed by the lead proj_v calls)
            dma(wvs[k][:], wqkv[k * P : (k + 1) * P, 2 * FG :])
        for k in range(KH):
            dma(xts[k][1][:], xT[k * P : (k + 1) * P, 2 * NQ :])
        for kk in range(2):
            dma(wps[kk][:], wp[kk * P : (kk + 1) * P, :])

        # PE warmup: junk matmuls to ramp the p-state while DMA lands.
        # jw[0] rotates the junk psum tag; extra junk is interleaved into
        # the lead-in so the HAM never sees a >3.4us idle window there.
        jw = [0]

        def junk_mm(tag="fill"):
            pw = ps.tile([P, NQ], fp32, tag=tag, name=f"warm{jw[0]}")
            nc.tensor.matmul(pw[:], junk[:, 0:P], junk[:], start=True, stop=True)
            jw[0] += 1

        for w in range(16):
            junk_mm()

        # ---- building blocks ----
        def proj_qk(m, jpair, on_act=False, lead_junk=False, jjs=(0, 1)):
            """q/k projection for feature chunk m, j-blocks 2*jpair+jjs.
            Weights per (m,k) load once and serve the jj matmuls. Bias-add
            copies go to ACT only during the lead-in (idle then); fillers
            use DVE so they never wedge between exps in the ACT FIFO.
            lead_junk: pad each chunk with a junk matmul so per-chunk DMA
            waits never accumulate into a HAM idle window."""
            pj = {
                jj: ps.tile([P, NQ], fp32, tag="fill", name=f"qk{m}_{jpair}_{jj}")
                for jj in jjs
            }
            for k in range(KH):
                for jj in jjs:
                    nc.tensor.matmul(
                        pj[jj][:],
                        wqks[k][:, m * P : (m + 1) * P],
                        xts[k][jpair][:, jj * NQ : (jj + 1) * NQ],
                        start=(k == 0),
                        stop=(k == KH - 1),
                    )
                if lead_junk:
                    junk_mm(tag="pvd")
            for jj in jjs:
                j = 2 * jpair + jj
                if on_act:
                    nc.scalar.activation(
                        qks[m][j][:], pj[jj][:], Ident, bias=bqk_sb[:, m : m + 1]
                    )
                else:
                    nc.vector.tensor_scalar_add(
                        qks[m][j][:], pj[jj][:], bqk_sb[:, m : m + 1]
                    )

        def proj_v(t, on_act=False):
            """v for token chunk t -> vts[t] cols [64h,64h+64) per head.
            The psum->SBUF cast rides on ACT in the PE-bound front phase."""
            pv = ps.tile([P, NQ], fp32, tag="fill", name=f"v{t}")
            th, tl = divmod(t, 8)
            for k in range(KH):
                nc.tensor.matmul(
                    pv[:, 0:FG],
                    xts[k][th][:, tl * P : (tl + 1) * P],
                    wvs[k][:],
                    start=(k == 0),
                    stop=(k == KH - 1),
                )
            v4 = vts[t].rearrange("p (h c) -> p h c", h=4, c=P)
            src = pv[:, 0:FG].rearrange("p (h c) -> p h c", h=4, c=64)
            if on_act:
                nc.scalar.copy(v4[:, :, 0:64], src)
            else:
                nc.vector.tensor_copy(v4[:, :, 0:64], src)

        def cproj(j, mql, tail=False, ptag="fill"):
            """c_proj for token rows j*512+128*mql. In the post-exp tail the
            psum->fp16 copies split across ACT (idle by then) and DVE, and
            ptag alternates over the attention psum tags (free by then) so
            consecutive cprojs don't serialize on fill-slot release."""
            pcs = [
                ps.tile([P, NQ], fp32, tag=ptag, name=f"c{j}_{mql}_{n}")
                for n in range(2)
            ]
            for kk in range(2):
                for n in range(2):
                    nc.tensor.matmul(
                        pcs[n][:],
                        ys[kk][j][:, mql * P : (mql + 1) * P],
                        wps[kk][:, n * NQ : (n + 1) * NQ],
                        start=(kk == 0),
                        stop=(kk == 1),
                    )
            r0 = j * NQ + mql * P
            for n in range(2):
                ot = outp.tile([P, NQ], fp16, tag="o", name=f"ot{j}_{mql}_{n}")
                if tail and n == 0:
                    nc.scalar.copy(ot[:], pcs[n][:])
                else:
                    nc.vector.tensor_copy(ot[:], pcs[n][:])
                nc.sync.dma_start(out[r0 : r0 + P, n * NQ : (n + 1) * NQ], ot[:])

        # ---- filler schedule: callables interleaved into group streams ----
        # Group order runs all pair-0 heads first, then pair-1, so the
        # pair-1 projections spread across the pair-0 groups and the cproj
        # work spreads across pair-1 groups: every group carries ~3.5us of
        # filler PE work instead of the front two carrying all of it.
        def F(fn, *a, **kw):
            return lambda: fn(*a, **kw)

        GROUP_ORDER = [(0, 0), (1, 0), (2, 0), (3, 0),
                       (0, 1), (1, 1), (2, 1), (3, 1)]
        # cproj fillers start at step 8 (foffset) so the producing group's
        # normalize chain has long drained before the cproj LDWEIGHTS waits
        # on ys -- a step-0 cproj stalls the in-order PE ~2.5us.
        fillers = {
            (0, 0): ([F(proj_qk, 2, 1)]
                     + [F(proj_v, t, on_act=True) for t in range(8, 16)], 0),
            (1, 0): ([F(proj_qk, 0, 1)], 0),
            (2, 0): ([F(proj_qk, 3, 0)], 0),
            (3, 0): ([F(proj_qk, 3, 1), F(proj_qk, 1, 0, jjs=(0,))], 0),
            (0, 1): ([F(proj_qk, 1, 0, jjs=(1,)), F(proj_qk, 1, 1)], 0),
            (1, 1): ([F(cproj, 0, q) for q in range(4)], 8),
            (2, 1): ([F(cproj, 1, q) for q in range(4)], 8),
            (3, 1): ([F(cproj, 2, q) for q in range(4)], 8),
        }
        # i-steps whose exp runs on DVE (per group): used in ACT-bound groups
        DVE_EXP = {g: DVE_EXP_I.get(g, ()) for g in GROUP_ORDER}

        def emit_s(j, p, i):
            sp = ps.tile([P, 2 * NQ], fp32, tag="s", name=f"s{j}{p}{i}")
            jb, ib = divmod(i, 4)
            for hh in range(2):
                nc.tensor.matmul(
                    sp[:, hh * NQ : (hh + 1) * NQ],
                    qks[2 + p][jb][64 * hh : 64 * hh + 64, ib * P : (ib + 1) * P],
                    qks[p][j][64 * hh : 64 * hh + 64, :],
                    start=True,
                    stop=True,
                    tile_position=(64 * hh, 0),
                )
            return sp

        def attend(j, p, pre, fo=0, first_sps=None, next_group=None, last=False):
            """16 i-steps for (j, pair p); pre = fillers list starting at
            step fo. S(0) and S(1) of the NEXT group are emitted during the
            last two steps so the exp stream never drains across group
            boundaries (the S pipeline stays two tiles deep)."""
            pvd = [
                ps.tile([P, NQ], fp32, tag="pvd", name=f"pvd{j}{p}{hh}")
                for hh in range(2)
            ]

            def emit_exp(i, sp):
                e = epool.tile([P, 2 * NQ], bf16, tag="e", name=f"e{j}{p}{i}")
                if i in DVE_EXP[(j, p)]:
                    scr = scrp.tile([P, 2 * NQ], fp32, tag="scr", name=f"sc{j}{p}{i}")
                    nc.vector._custom_dve(
                        EXP_BASE, out=scr[:], in0=sp[:],
                        s0=1.0 / 256, s1=1.0, imm2=0.5,
                    )
                    nc.vector._custom_dve(EXP_SQ8, out=e[:], in0=scr[:])
                else:
                    nc.scalar.activation(e[:], sp[:], Exp, bias=bias0[:, 0:1])
                return e

            # software-pipelined emission: the S pipeline runs two tiles
            # ahead of the exps (S(i+2) emitted at step i), and PV(i) is
            # DEFERRED one step (three for DVE exps, whose 2-op latency is
            # ~2.5us, and for i=0, which would otherwise block the in-order
            # PE on the previous group's pvd-freeing pvs copies).
            nf = len(pre)
            fi = 0
            sps = list(first_sps) if first_sps is not None else [
                emit_s(j, p, 0), emit_s(j, p, 1)]
            next_sps = []
            pend = []  # (i, e_tile, due_step), ordered by i

            def emit_pv(i0, e0):
                for hh in range(2):
                    nc.tensor.matmul(
                        pvd[hh][:],
                        vts[i0][:, (2 * p + hh) * P : (2 * p + hh + 1) * P],
                        e0[:, hh * NQ : (hh + 1) * NQ],
                        start=(i0 == 0),
                        stop=(i0 == NI - 1),
                    )

            def flush(cur):
                while pend and pend[0][2] <= cur:
                    i0, e0, _ = pend.pop(0)
                    emit_pv(i0, e0)

            for i in range(NI):
                e = emit_exp(i, sps[i])
                if i + 2 < NI:
                    sps.append(emit_s(j, p, i + 2))
                elif next_group is not None:
                    next_sps.append(emit_s(next_group[0], next_group[1], i + 2 - NI))
                due = i + 1
                if i == 0 or i in DVE_EXP[(j, p)]:
                    due = i + 3
                pend.append((i, e, due))
                flush(i)
                while fi < nf and i >= fo and fi * (NI - fo) <= (i - fo) * nf:
                    pre[fi]()
                    fi += 1
            while fi < nf:
                pre[fi]()
                fi += 1
            flush(NI + 3)
            # normalize: 1/denom from psum row 64, broadcast, multiply.
            # pvd psum is needed by the NEXT group's PVs, so the pvd reads
            # (dh + pvs copies) all run before the recip/broadcast/mul
            # chain. In the tail (last) the copies ride the then-idle ACT.
            dhs, pvss = [], []
            cp = nc.scalar.copy if last else nc.vector.tensor_copy
            for hh in range(2):
                dh = rcpp.tile([1, NQ], fp32, tag="d", name=f"dh{j}{p}{hh}")
                # custom DVE ops don't shift partitions: stage row 64 to p0
                cp(dh[:], pvd[hh][64:65, :])
                pvs = pvsp.tile([64, NQ], fp32, tag="pvs", name=f"pvs{j}{p}{hh}")
                cp(pvs[:], pvd[hh][0:64, :])
                dhs.append(dh)
                pvss.append(pvs)
            for hh in range(2):
                rh = rcpp.tile([1, NQ], fp32, tag="r", name=f"rh{j}{p}{hh}")
                nc.vector.reciprocal_approx_fast(out=rh[:], in_=dhs[hh][:])
                bc = rcpp.tile([64, NQ], fp32, tag="b", name=f"bc{j}{p}{hh}")
                nc.gpsimd.partition_broadcast(bc[:], rh[0:1, :])
                nc.vector.tensor_mul(
                    ys[p][j][64 * hh : 64 * hh + 64, :], pvss[hh][:], bc[:]
                )
            return next_sps

        # ---- main schedule ----
        proj_qk(2, 0, on_act=True, lead_junk=True)  # k pair0 j01
        proj_qk(0, 0, on_act=True, lead_junk=True)  # q pair0 j01
        for t in range(8):           # first-half v chunks ride the DMA shadow
            proj_v(t, on_act=True)
            if t < 4:
                junk_mm(tag="pvd")
        sps = None
        for gi, (j, p) in enumerate(GROUP_ORDER):
            nxt = GROUP_ORDER[gi + 1] if gi + 1 < len(GROUP_ORDER) else None
            pre, fo = fillers[(j, p)]
            sps = attend(j, p, pre, fo=fo, first_sps=sps, next_group=nxt,
                         last=(nxt is None))
        # tail: junk matmuls keep HAM warm while the last normalize drains
        for w in range(12):
            junk_mm()
        for q in range(4):
            cproj(3, q, tail=True, ptag=("s" if q % 2 else "pvd"))


def _get_nc():
    if "nc" not in _CACHE:
        _CACHE["nc"] = _build()
    return _CACHE["nc"]


def _make_in_maps(x, W_attn, b_attn, W_proj):
    import ml_dtypes

    bf = ml_dtypes.bfloat16
    x = np.asarray(x, np.float32)
    W_attn = np.asarray(W_attn, np.float32)
    b_attn = np.asarray(b_attn, np.float32)
    scale = 1.0 / np.sqrt(np.float32(HD))
    W_proj = np.asarray(W_proj, np.float32)
    in_maps = []
    for c in range(NCORES):
        b, g = divmod(c, 4)
        sl = slice(FG * g, FG * (g + 1))
        wq = W_attn[:, sl] * scale
        wk = W_attn[:, H:][:, sl]
        wv = W_attn[:, 2 * H :][:, sl]
        in_maps.append(
            {
                "xT": np.ascontiguousarray(x[b].T).astype(bf),
                "wqkv": np.ascontiguousarray(
                    np.concatenate([wq, wk, wv], axis=1)
                ).astype(bf),
                "bqk": np.concatenate(
                    [b_attn[sl] * scale, b_attn[H:][sl]]
                ).astype(np.float32),
                "wp": np.ascontiguousarray(W_proj[sl, :]).astype(bf),
            }
        )
    return in_maps


def _gather(results, b_attn, W_proj, b_proj):
    b_attn = np.asarray(b_attn, np.float64)
    W_proj = np.asarray(W_proj, np.float64)
    b_proj = np.asarray(b_proj, np.float64)
    # v-bias commutes through softmax: y = sum_k p_k (v_k + bv) = y0 + bv
    host_bias = (b_attn[2 * H :] @ W_proj + b_proj).astype(np.float32)
    y = np.empty((B, T, H), np.float32)
    for b in range(B):
        acc = results[4 * b]["out"].astype(np.float32)
        for g in range(1, 4):
            acc = acc + results[4 * b + g]["out"].astype(np.float32)
        y[b] = acc + host_bias[None, :]
    return y


def run(x, W_attn, b_attn, W_proj, b_proj, trace=False):
    from concourse.bass_utils import run_bass_kernel_spmd

    nc = _get_nc()
    in_maps = _make_in_maps(x, W_attn, b_attn, W_proj)
    res = run_bass_kernel_spmd(nc, in_maps, list(range(NCORES)), trace=trace)
    return _gather(res.results, b_attn, W_proj, b_proj), res


def kernel(x, W_attn, b_attn, W_proj, b_proj):
    y, _ = run(x, W_attn, b_attn, W_proj, b_proj, trace=False)
    return y



# revision 44
# speedup vs baseline: 1.1679x; 1.1679x over previous
"""Causal self-attention (non-masked softmax) for TRN2, 8 NeuronCores.

Sharding: 2-way data parallel over batch x 4-way tensor parallel over heads.
Core c: batch c//4, head group c%4 (4 heads, 256 features). Host sums the
4 row-parallel c_proj partials per batch (fp16 partials) and adds the
host-folded biases (b_proj + b_attn_v @ W_proj; v-bias commutes through
softmax since sum_k p_k = 1).

Engine split per core:
  PE:  QKV projections, S^T (row-packed head pairs), PV (v augmented with a
       ones column so psum row 64 is the softmax denominator), c_proj.
  ACT: exp over S^T psum tiles [128,1024] (~1.15us each; with the PE's
       ~137us of matmuls at 2.4GHz the two streams are nearly balanced),
       plus v/qk staging copies in the PE-bound lead-in phase.
  DVE: one exp tile per ACT-bound group via two custom ops (compound base
       (1+s/256+s^2/(2*256^2)) then 8 chained squarings; ~6e-3 rel err at
       bf16 out), qk bias staging, denominator reciprocal, y normalize
       muls, c_proj psum->fp16 staging.
  GPSIMD: partition_broadcast of 1/denom rows, input DMA issue.

PSUM: tag s = 2x[128,1024] (S tiles, double buffered), pvd = 2x[128,512]
(PV accumulators for the in-flight head pair), fill = 2x[128,512]
(projection / c_proj scratch) = 8 banks total.

Schedule: 8 attention groups (j, pair) ordered all-pair0-then-all-pair1 so
the pair-1 q/k projections spread as fillers over the pair-0 groups and
cproj(j) spreads over the pair-1 groups; every group carries ~3.5us of
filler PE work against its 18.4us exp stream. Steady state per step: the
S pipeline runs two tiles ahead of the exp stream (S(i+2) emitted at step
i; S(0)/S(1) of the next group pre-emitted during the last two steps) and
PV(i) is deferred one step (three for DVE-exp steps and i=0) so the
in-order PE never blocks on exp completion or on the previous group's
pvd-freeing copies. Fillers are split into <=1.7us pieces and paced into
the back half of ACT-bound groups where the PE has accumulated slack.
Junk matmuls bridge the DMA-bound lead-in and the normalize tail so the
PE HAM clock gate stays at 2.4GHz for the whole kernel.
"""

import numpy as np

B, T, H, NH, HD = 2, 2048, 1024, 16, 64
P, FG = 128, 256
NQ = 512          # query block per j
NJ = 4            # T/NQ
NI = 16           # key chunks of 128
KH = 8            # hidden chunks of 128
W3 = 3 * FG       # wqkv row width per core (768)
NCORES = 8

# i-steps per (j,pair) group whose exp runs on DVE instead of ACT
# (only in groups where the ACT exp stream, not the PE, is the limiter)
DVE_EXP_I = {
    (1, 0): (5, 10),
    (2, 0): (5, 10),
    (3, 0): (5, 10),
    (0, 1): (5,),
    (1, 1): (5,),
    (2, 1): (5,),
    (3, 1): (5,),
}

_CACHE = {}


def _register_dve_exp():
    """Register the custom DVE exp ops (idempotent). exp(s) ~= base^(256)
    with base = 1 + s/256 + (s/256)^2/2: op1 computes the base, op2 squares
    eight times. Validated on hw: max rel err ~6e-3 over s in [-8, 8]."""
    import concourse.dve_ops as dve_ops
    from concourse.dve_ops import DveOp, OPS, CUSTOM_DVE_SPECS, _SUB_OPCODE_FOR_NAME
    from concourse.dve_spec import Spec, Src0, C0, C1, C2, lower, sq
    from concourse.dve_uop import DveOpSpec

    have = {o.name: o for o in OPS}
    if "EXP_BASE_XK" in have:
        return have["EXP_BASE_XK"], have["EXP_SQ8_XK"]

    def make(name, body, ref):
        spec = Spec(body=body, reference=ref)
        row = max(_SUB_OPCODE_FOR_NAME.values()) + 1
        _SUB_OPCODE_FOR_NAME[name] = row
        shas = {}
        for ver in ("v3",):
            tmp = DveOpSpec(name=name, opcode=row, uops=lower(spec, ver=ver),
                            rd1_en=False)
            shas[ver] = tmp.sha(ver)
        op = DveOp(name, spec, subdim=False, uops_sha=shas)
        OPS.append(op)
        CUSTOM_DVE_SPECS[name] = spec
        return op

    w = Src0 * C0
    base_body = C1 + w * (C1 + w * C2)

    def ref_base(in0, in1, s0, s1, imm2):
        ww = in0.astype(np.float32) * np.float32(s0)
        return (np.float32(s1) + ww * (np.float32(s1) + ww * np.float32(imm2))
                ).astype(np.float32)

    x = Src0
    for _ in range(8):
        x = sq(x)

    def ref_sq8(in0, in1, s0, s1, imm2):
        v = in0.astype(np.float32)
        for _ in range(8):
            v = (v * v).astype(np.float32)
        return v

    return make("EXP_BASE_XK", base_body, ref_base), make("EXP_SQ8_XK", x, ref_sq8)


def _build():
    import concourse.bacc as bacc
    import concourse.mybir as mybir
    import concourse.tile as tile

    EXP_BASE, EXP_SQ8 = _register_dve_exp()

    fp32 = mybir.dt.float32
    fp16 = mybir.dt.float16
    bf16 = mybir.dt.bfloat16

    nc = bacc.Bacc("TRN2", debug=False)
    xT = nc.dram_tensor("xT", [H, T], bf16, kind="ExternalInput").ap()
    wqkv = nc.dram_tensor("wqkv", [H, W3], bf16, kind="ExternalInput").ap()
    bqk = nc.dram_tensor("bqk", [2 * FG], fp32, kind="ExternalInput").ap()
    wp = nc.dram_tensor("wp", [FG, H], bf16, kind="ExternalInput").ap()
    out = nc.dram_tensor("out", [T, H], fp16, kind="ExternalOutput").ap()

    with tile.TileContext(nc) as tc:
        _emit(nc, tc, mybir, EXP_BASE, EXP_SQ8, xT, wqkv, bqk, wp, out)
    nc.compile()
    return nc


def _emit(nc, tc, mybir, EXP_BASE, EXP_SQ8, xT, wqkv, bqk, wp, out):
    from contextlib import ExitStack

    fp32 = mybir.dt.float32
    fp16 = mybir.dt.float16
    bf16 = mybir.dt.bfloat16
    Exp = mybir.ActivationFunctionType.Exp
    Ident = mybir.ActivationFunctionType.Identity

    with ExitStack() as ctx:
        pool = lambda name, bufs=1, space="SBUF": ctx.enter_context(
            tc.tile_pool(name=name, bufs=bufs, space=space)
        )

        # ---- persistent SBUF tiles (fine-grained so deps stay fine) ----
        const = pool("const")
        bias0 = const.tile([P, 1], fp32)
        nc.vector.memset(bias0[:], 0.0)
        junk = const.tile([P, NQ], bf16)        # PE warmup operand
        nc.vector.memset(junk[:], 0.001)
        warm = const.tile([P, 8], fp32)
        nc.vector.memset(warm[:], 0.0)
        bqk_sb = const.tile([P, 4], fp32)       # bias col per m-chunk
        nc.sync.dma_start(bqk_sb[:], bqk.rearrange("(m p) -> p m", p=P))

        xtp = pool("xt")
        # per (hidden chunk, token half) so early readers don't wait all DMA
        xts = [[xtp.tile([P, T // 2], bf16, name=f"xt{k}_{h}") for h in range(2)] for k in range(KH)]
        wqp = pool("wq")
        # q/k and v weight columns in separate tiles so the critical q/k
        # prefix DMA isn't diluted by v columns
        wqks = [wqp.tile([P, 2 * FG], bf16, name=f"wqk{k}") for k in range(KH)]
        wvs = [wqp.tile([P, FG], bf16, name=f"wv{k}") for k in range(KH)]
        wpp = pool("wpp")
        wps = [wpp.tile([P, H], bf16, name=f"wp{k}") for k in range(2)]
        qkp = pool("qk")
        qks = [[qkp.tile([P, NQ], bf16, name=f"qk{m}_{j}") for j in range(NJ)] for m in range(4)]
        vp = pool("v")
        vts = [vp.tile([P, 4 * P], bf16, name=f"v{t}") for t in range(NI)]
        yp = pool("y")
        ys = [[yp.tile([P, NQ], bf16, name=f"y{k}_{j}") for j in range(NJ)] for k in range(2)]

        # ---- working pools ----
        epool = pool("e", bufs=4)
        scrp = pool("scr", bufs=2)      # DVE exp fp32 intermediate
        pvsp = pool("pvs", bufs=4)      # staged PV numerators
        rcpp = pool("rcp", bufs=4)      # 1/denom rows + broadcasts
        outp = pool("o", bufs=4)        # fp16 out staging
        ps = ctx.enter_context(tc.tile_pool(name="ps", bufs=2, space="PSUM"))

        # ---- warmups (off critical path): ACT exp table, PE p-state ----
        nc.scalar.activation(warm[:], warm[:], Exp, bias=bias0[:, 0:1])
        for t in range(NI):  # mark ones col + zero pad of v tiles
            v4 = vts[t].rearrange("p (h c) -> p h c", h=4, c=P)
            nc.vector.memset(v4[:, :, 64:66], 0.0)
            nc.vector.memset(v4[:, :, 64:65], 1.0)

        # ---- input DMAs, spread across idle engine queues (not ACT:
        # scalar-queue DMA issues would steal ~6us from the exp stream) ----
        dq = [nc.sync, nc.gpsimd]
        di = [0]

        def dma(dst, src):
            dq[di[0] % 2].dma_start(dst, src)
            di[0] += 1

        for k in range(KH):  # critical prefix: q/k weights + first token half
            dma(wqks[k][:], wqkv[k * P : (k + 1) * P, 0 : 2 * FG])
            dma(xts[k][0][:], xT[k * P : (k + 1) * P, 0 : 2 * NQ])
        for k in range(KH):  # v weights (needed by the lead proj_v calls)
            dma(wvs[k][:], wqkv[k * P : (k + 1) * P, 2 * FG :])
        for k in range(KH):
            dma(xts[k][1][:], xT[k * P : (k + 1) * P, 2 * NQ :])
        for kk in range(2):
            dma(wps[kk][:], wp[kk * P : (kk + 1) * P, :])

        # PE warmup: junk matmuls to ramp the p-state while DMA lands.
        # jw[0] rotates the junk psum tag; extra junk is interleaved into
        # the lead-in so the HAM never sees a >3.4us idle window there.
        jw = [0]

        def junk_mm(tag="fill"):
            pw = ps.tile([P, NQ], fp32, tag=tag, name=f"warm{jw[0]}")
            nc.tensor.matmul(pw[:], junk[:, 0:P], junk[:], start=True, stop=True)
            jw[0] += 1

        for w in range(16):
            junk_mm()

        # ---- building blocks ----
        def proj_qk(m, jpair, on_act=False, lead_junk=False, jjs=(0, 1)):
            """q/k projection for feature chunk m, j-blocks 2*jpair+jjs.
            Weights per (m,k) load once and serve the jj matmuls. Bias-add
            copies go to ACT only during the lead-in (idle then); fillers
            use DVE so they never wedge between exps in the ACT FIFO.
            lead_junk: pad each chunk with a junk matmul so per-chunk DMA
            waits never accumulate into a HAM idle window."""
            pj = {
                jj: ps.tile([P, NQ], fp32, tag="fill", name=f"qk{m}_{jpair}_{jj}")
                for jj in jjs
            }
            for k in range(KH):
                for jj in jjs:
                    nc.tensor.matmul(
                        pj[jj][:],
                        wqks[k][:, m * P : (m + 1) * P],
                        xts[k][jpair][:, jj * NQ : (jj + 1) * NQ],
                        start=(k == 0),
                        stop=(k == KH - 1),
                    )
                if lead_junk:
                    junk_mm(tag="pvd")
            for jj in jjs:
                j = 2 * jpair + jj
                if on_act:
                    nc.scalar.activation(
                        qks[m][j][:], pj[jj][:], Ident, bias=bqk_sb[:, m : m + 1]
                    )
                else:
                    nc.vector.tensor_scalar_add(
                        qks[m][j][:], pj[jj][:], bqk_sb[:, m : m + 1]
                    )

        def proj_v(t, on_act=False):
            """v for token chunk t -> vts[t] cols [64h,64h+64) per head.
            The psum->SBUF cast rides on ACT in the PE-bound front phase."""
            pv = ps.tile([P, NQ], fp32, tag="fill", name=f"v{t}")
            th, tl = divmod(t, 8)
            for k in range(KH):
                nc.tensor.matmul(
                    pv[:, 0:FG],
                    xts[k][th][:, tl * P : (tl + 1) * P],
                    wvs[k][:],
                    start=(k == 0),
                    stop=(k == KH - 1),
                )
            v4 = vts[t].rearrange("p (h c) -> p h c", h=4, c=P)
            src = pv[:, 0:FG].rearrange("p (h c) -> p h c", h=4, c=64)
            if on_act:
                nc.scalar.copy(v4[:, :, 0:64], src)
            else:
                nc.vector.tensor_copy(v4[:, :, 0:64], src)

        def cproj(j, mql, tail=False, ptag="fill"):
            """c_proj for token rows j*512+128*mql. In the post-exp tail the
            psum->fp16 copies split across ACT (idle by then) and DVE, and
            ptag alternates over the attention psum tags (free by then) so
            consecutive cprojs don't serialize on fill-slot release."""
            pcs = [
                ps.tile([P, NQ], fp32, tag=ptag, name=f"c{j}_{mql}_{n}")
                for n in range(2)
            ]
            for kk in range(2):
                for n in range(2):
                    nc.tensor.matmul(
                        pcs[n][:],
                        ys[kk][j][:, mql * P : (mql + 1) * P],
                        wps[kk][:, n * NQ : (n + 1) * NQ],
                        start=(kk == 0),
                        stop=(kk == 1),
                    )
            r0 = j * NQ + mql * P
            for n in range(2):
                ot = outp.tile([P, NQ], fp16, tag="o", name=f"ot{j}_{mql}_{n}")
                if tail and n == 0:
                    nc.scalar.copy(ot[:], pcs[n][:])
                else:
                    nc.vector.tensor_copy(ot[:], pcs[n][:])
                nc.sync.dma_start(out[r0 : r0 + P, n * NQ : (n + 1) * NQ], ot[:])

        # ---- filler schedule: callables interleaved into group streams ----
        # Group order runs all pair-0 heads first, then pair-1, so the
        # pair-1 projections spread across the pair-0 groups and the cproj
        # work spreads across pair-1 groups: every group carries ~3.5us of
        # filler PE work instead of the front two carrying all of it.
        def F(fn, *a, **kw):
            return lambda: fn(*a, **kw)

        GROUP_ORDER = [(0, 0), (1, 0), (2, 0), (3, 0),
                       (0, 1), (1, 1), (2, 1), (3, 1)]
        # cproj fillers start at step 8 (foffset) so the producing group's
        # normalize chain has long drained before the cproj LDWEIGHTS waits
        # on ys -- a step-0 cproj stalls the in-order PE ~2.5us.
        fillers = {
            (0, 0): ([F(proj_qk, 2, 0, jjs=(1,)), F(proj_qk, 2, 1),
                      F(proj_qk, 0, 0, jjs=(1,))]
                     + [F(proj_v, t, on_act=True) for t in range(8, 16)], 0),
            (1, 0): ([F(proj_qk, 0, 1)], 0),
            (2, 0): ([F(proj_qk, 3, 0)], 0),
            (3, 0): ([F(proj_qk, 3, 1), F(proj_qk, 1, 0, jjs=(0,))], 0),
            (0, 1): ([F(proj_qk, 1, 0, jjs=(1,)), F(proj_qk, 1, 1)], 0),
            (1, 1): ([F(cproj, 0, q) for q in range(4)], 8),
            (2, 1): ([F(cproj, 1, q) for q in range(4)], 8),
            (3, 1): ([F(cproj, 2, q) for q in range(4)], 8),
        }
        # i-steps whose exp runs on DVE (per group): used in ACT-bound groups
        DVE_EXP = {g: DVE_EXP_I.get(g, ()) for g in GROUP_ORDER}

        def emit_s(j, p, i):
            sp = ps.tile([P, 2 * NQ], fp32, tag="s", name=f"s{j}{p}{i}")
            jb, ib = divmod(i, 4)
            for hh in range(2):
                nc.tensor.matmul(
                    sp[:, hh * NQ : (hh + 1) * NQ],
                    qks[2 + p][jb][64 * hh : 64 * hh + 64, ib * P : (ib + 1) * P],
                    qks[p][j][64 * hh : 64 * hh + 64, :],
                    start=True,
                    stop=True,
                    tile_position=(64 * hh, 0),
                )
            return sp

        def attend(j, p, pre, fo=0, first_sps=None, next_group=None, last=False):
            """16 i-steps for (j, pair p); pre = fillers list starting at
            step fo. S(0) and S(1) of the NEXT group are emitted during the
            last two steps so the exp stream never drains across group
            boundaries (the S pipeline stays two tiles deep)."""
            pvd = [
                ps.tile([P, NQ], fp32, tag="pvd", name=f"pvd{j}{p}{hh}")
                for hh in range(2)
            ]

            def emit_exp(i, sp):
                e = epool.tile([P, 2 * NQ], bf16, tag="e", name=f"e{j}{p}{i}")
                if i in DVE_EXP[(j, p)]:
                    scr = scrp.tile([P, 2 * NQ], fp32, tag="scr", name=f"sc{j}{p}{i}")
                    nc.vector._custom_dve(
                        EXP_BASE, out=scr[:], in0=sp[:],
                        s0=1.0 / 256, s1=1.0, imm2=0.5,
                    )
                    nc.vector._custom_dve(EXP_SQ8, out=e[:], in0=scr[:])
                else:
                    nc.scalar.activation(e[:], sp[:], Exp, bias=bias0[:, 0:1])
                return e

            # software-pipelined emission: the S pipeline runs two tiles
            # ahead of the exps (S(i+2) emitted at step i), and PV(i) is
            # DEFERRED one step (three for DVE exps, whose 2-op latency is
            # ~2.5us, and for i=0, which would otherwise block the in-order
            # PE on the previous group's pvd-freeing pvs copies).
            nf = len(pre)
            fi = 0
            sps = list(first_sps) if first_sps is not None else [
                emit_s(j, p, 0), emit_s(j, p, 1)]
            next_sps = []
            pend = []  # (i, e_tile, due_step), ordered by i

            def emit_pv(i0, e0):
                for hh in range(2):
                    nc.tensor.matmul(
                        pvd[hh][:],
                        vts[i0][:, (2 * p + hh) * P : (2 * p + hh + 1) * P],
                        e0[:, hh * NQ : (hh + 1) * NQ],
                        start=(i0 == 0),
                        stop=(i0 == NI - 1),
                    )

            def flush(cur):
                while pend and pend[0][2] <= cur:
                    i0, e0, _ = pend.pop(0)
                    emit_pv(i0, e0)

            for i in range(NI):
                e = emit_exp(i, sps[i])
                if i + 2 < NI:
                    sps.append(emit_s(j, p, i + 2))
                elif next_group is not None:
                    next_sps.append(emit_s(next_group[0], next_group[1], i + 2 - NI))
                due = i + 1
                if i == 0 or i in DVE_EXP[(j, p)]:
                    due = i + 3
                pend.append((i, e, due))
                flush(i)
                while fi < nf and i >= fo and fi * (NI - fo) <= (i - fo) * nf:
                    pre[fi]()
                    fi += 1
            while fi < nf:
                pre[fi]()
                fi += 1
            flush(NI + 3)
            # normalize: 1/denom from psum row 64, broadcast, multiply.
            # pvd psum is needed by the NEXT group's PVs, so the pvd reads
            # (dh + pvs copies) all run before the recip/broadcast/mul
            # chain. In the tail (last) the copies ride the then-idle ACT.
            dhs, pvss = [], []
            cp = nc.scalar.copy if last else nc.vector.tensor_copy
            for hh in range(2):
                dh = rcpp.tile([1, NQ], fp32, tag="d", name=f"dh{j}{p}{hh}")
                # custom DVE ops don't shift partitions: stage row 64 to p0
                cp(dh[:], pvd[hh][64:65, :])
                pvs = pvsp.tile([64, NQ], fp32, tag="pvs", name=f"pvs{j}{p}{hh}")
                cp(pvs[:], pvd[hh][0:64, :])
                dhs.append(dh)
                pvss.append(pvs)
            for hh in range(2):
                rh = rcpp.tile([1, NQ], fp32, tag="r", name=f"rh{j}{p}{hh}")
                nc.vector.reciprocal_approx_fast(out=rh[:], in_=dhs[hh][:])
                bc = rcpp.tile([64, NQ], fp32, tag="b", name=f"bc{j}{p}{hh}")
                nc.gpsimd.partition_broadcast(bc[:], rh[0:1, :])
                nc.vector.tensor_mul(
                    ys[p][j][64 * hh : 64 * hh + 64, :], pvss[hh][:], bc[:]
                )
            return next_sps

        # ---- main schedule ----
        # lead-in projects only the j=0 halves of k/q pair0 -- the minimum
        # for exp(0) -- so the exp stream starts ~3.4us earlier; the j=1
        # halves run as the first fillers of group (0,0).
        proj_qk(2, 0, on_act=True, lead_junk=True, jjs=(0,))  # k pair0 j0
        proj_qk(0, 0, on_act=True, lead_junk=True, jjs=(0,))  # q pair0 j0
        for t in range(8):           # first-half v chunks ride the DMA shadow
            proj_v(t, on_act=True)
            if t < 4:
                junk_mm(tag="pvd")
        sps = None
        for gi, (j, p) in enumerate(GROUP_ORDER):
            nxt = GROUP_ORDER[gi + 1] if gi + 1 < len(GROUP_ORDER) else None
            pre, fo = fillers[(j, p)]
            sps = attend(j, p, pre, fo=fo, first_sps=sps, next_group=nxt,
                         last=(nxt is None))
        # tail: all four cproj(3,*) accumulate their pair-0 half right away
        # (ys[0][3] has been ready since group (3,0)); the pair-1 half and
        # the copies/DMAs follow once the last normalize lands. All 8 psum
        # banks are free here, so the four cprojs live concurrently across
        # the s/fill/pvd tags ("pvd" last: its slots free only after the
        # normalize's pvd reads). Junk matmuls bridge the remaining wait.
        for w in range(6):
            junk_mm()
        tails = {}
        TAIL_ORDER = [(0, "s"), (1, "s"), (3, "fill"), (2, "pvd")]
        for q, tag in TAIL_ORDER:
            if tag == "s":
                pc = ps.tile([P, 2 * NQ], fp32, tag="s", name=f"ct{q}")
                pcs = [pc[:, 0:NQ], pc[:, NQ : 2 * NQ]]
            else:
                pcs = [
                    ps.tile([P, NQ], fp32, tag=tag, name=f"ct{q}_{n}")[:]
                    for n in range(2)
                ]
            for n in range(2):
                nc.tensor.matmul(
                    pcs[n],
                    ys[0][3][:, q * P : (q + 1) * P],
                    wps[0][:, n * NQ : (n + 1) * NQ],
                    start=True,
                    stop=False,
                )
            tails[q] = pcs
        for q, _ in TAIL_ORDER:
            pcs = tails[q]
            for n in range(2):
                nc.tensor.matmul(
                    pcs[n],
                    ys[1][3][:, q * P : (q + 1) * P],
                    wps[1][:, n * NQ : (n + 1) * NQ],
                    start=False,
                    stop=True,
                )
            r0 = 3 * NQ + q * P
            for n in range(2):
                ot = outp.tile([P, NQ], fp16, tag="o", name=f"ott{q}_{n}")
                if n == 0:
                    nc.scalar.copy(ot[:], pcs[n])
                else:
                    nc.vector.tensor_copy(ot[:], pcs[n])
                nc.sync.dma_start(out[r0 : r0 + P, n * NQ : (n + 1) * NQ], ot[:])


def _get_nc():
    if "nc" not in _CACHE:
        _CACHE["nc"] = _build()
    return _CACHE["nc"]


def _make_in_maps(x, W_attn, b_attn, W_proj):
    import ml_dtypes

    bf = ml_dtypes.bfloat16
    x = np.asarray(x, np.float32)
    W_attn = np.asarray(W_attn, np.float32)
    b_attn = np.asarray(b_attn, np.float32)
    scale = 1.0 / np.sqrt(np.float32(HD))
    W_proj = np.asarray(W_proj, np.float32)
    in_maps = []
    for c in range(NCORES):
        b, g = divmod(c, 4)
        sl = slice(FG * g, FG * (g + 1))
        wq = W_attn[:, sl] * scale
        wk = W_attn[:, H:][:, sl]
        wv = W_attn[:, 2 * H :][:, sl]
        in_maps.append(
            {
                "xT": np.ascontiguousarray(x[b].T).astype(bf),
                "wqkv": np.ascontiguousarray(
                    np.concatenate([wq, wk, wv], axis=1)
                ).astype(bf),
                "bqk": np.concatenate(
                    [b_attn[sl] * scale, b_attn[H:][sl]]
                ).astype(np.float32),
                "wp": np.ascontiguousarray(W_proj[sl, :]).astype(bf),
            }
        )
    return in_maps


def _gather(results, b_attn, W_proj, b_proj):
    b_attn = np.asarray(b_attn, np.float64)
    W_proj = np.asarray(W_proj, np.float64)
    b_proj = np.asarray(b_proj, np.float64)
    # v-bias commutes through softmax: y = sum_k p_k (v_k + bv) = y0 + bv
    host_bias = (b_attn[2 * H :] @ W_proj + b_proj).astype(np.float32)
    y = np.empty((B, T, H), np.float32)
    for b in range(B):
        acc = results[4 * b]["out"].astype(np.float32)
        for g in range(1, 4):
            acc = acc + results[4 * b + g]["out"].astype(np.float32)
        y[b] = acc + host_bias[None, :]
    return y


def run(x, W_attn, b_attn, W_proj, b_proj, trace=False):
    from concourse.bass_utils import run_bass_kernel_spmd

    nc = _get_nc()
    in_maps = _make_in_maps(x, W_attn, b_attn, W_proj)
    res = run_bass_kernel_spmd(nc, in_maps, list(range(NCORES)), trace=trace)
    return _gather(res.results, b_attn, W_proj, b_proj), res


def kernel(x, W_attn, b_attn, W_proj, b_proj):
    y, _ = run(x, W_attn, b_attn, W_proj, b_proj, trace=False)
    return y



# revision 46
# speedup vs baseline: 1.1728x; 1.0042x over previous
"""Causal self-attention (non-masked softmax) for TRN2, 8 NeuronCores.

Sharding: 2-way data parallel over batch x 4-way tensor parallel over heads.
Core c: batch c//4, head group c%4 (4 heads, 256 features). Host sums the
4 row-parallel c_proj partials per batch (fp16 partials) and adds the
host-folded biases (b_proj + b_attn_v @ W_proj; v-bias commutes through
softmax since sum_k p_k = 1).

Engine split per core:
  PE:  QKV projections, S^T (row-packed head pairs), PV (v augmented with a
       ones column so psum row 64 is the softmax denominator), c_proj.
  ACT: exp over S^T psum tiles [128,1024] (~1.15us each; with the PE's
       ~137us of matmuls at 2.4GHz the two streams are nearly balanced),
       plus v/qk staging copies in the PE-bound lead-in phase.
  DVE: one exp tile per ACT-bound group via two custom ops (compound base
       (1+s/256+s^2/(2*256^2)) then 8 chained squarings; ~6e-3 rel err at
       bf16 out), qk bias staging, denominator reciprocal, y normalize
       muls, c_proj psum->fp16 staging.
  GPSIMD: partition_broadcast of 1/denom rows, input DMA issue.

PSUM: tag s = 2x[128,1024] (S tiles, double buffered), pvd = 2x[128,512]
(PV accumulators for the in-flight head pair), fill = 2x[128,512]
(projection / c_proj scratch) = 8 banks total.

Schedule: 8 attention groups (j, pair) ordered all-pair0-then-all-pair1 so
the pair-1 q/k projections spread as fillers over the pair-0 groups and
cproj(j) spreads over the pair-1 groups; every group carries ~3.5us of
filler PE work against its 18.4us exp stream. Steady state per step: the
S pipeline runs two tiles ahead of the exp stream (S(i+2) emitted at step
i; S(0)/S(1) of the next group pre-emitted during the last two steps) and
PV(i) is deferred one step (three for DVE-exp steps and i=0) so the
in-order PE never blocks on exp completion or on the previous group's
pvd-freeing copies. Fillers are split into <=1.7us pieces and paced into
the back half of ACT-bound groups where the PE has accumulated slack.
Junk matmuls bridge the DMA-bound lead-in and the normalize tail so the
PE HAM clock gate stays at 2.4GHz for the whole kernel.
"""

import numpy as np

B, T, H, NH, HD = 2, 2048, 1024, 16, 64
P, FG = 128, 256
NQ = 512          # query block per j
NJ = 4            # T/NQ
NI = 16           # key chunks of 128
KH = 8            # hidden chunks of 128
W3 = 3 * FG       # wqkv row width per core (768)
NCORES = 8

# i-steps per (j,pair) group whose exp runs on DVE instead of ACT
# (only in groups where the ACT exp stream, not the PE, is the limiter)
DVE_EXP_I = {
    (1, 0): (5, 10),
    (2, 0): (5, 10),
    (3, 0): (5, 10),
    (0, 1): (5,),
    (1, 1): (5,),
    (2, 1): (5,),
    (3, 1): (5,),
}

_CACHE = {}


def _register_dve_exp():
    """Register the custom DVE exp ops (idempotent). exp(s) ~= base^(256)
    with base = 1 + s/256 + (s/256)^2/2: op1 computes the base, op2 squares
    eight times. Validated on hw: max rel err ~6e-3 over s in [-8, 8]."""
    import concourse.dve_ops as dve_ops
    from concourse.dve_ops import DveOp, OPS, CUSTOM_DVE_SPECS, _SUB_OPCODE_FOR_NAME
    from concourse.dve_spec import Spec, Src0, C0, C1, C2, lower, sq
    from concourse.dve_uop import DveOpSpec

    have = {o.name: o for o in OPS}
    if "EXP_BASE_XK" in have:
        return have["EXP_BASE_XK"], have["EXP_SQ8_XK"]

    def make(name, body, ref):
        spec = Spec(body=body, reference=ref)
        row = max(_SUB_OPCODE_FOR_NAME.values()) + 1
        _SUB_OPCODE_FOR_NAME[name] = row
        shas = {}
        for ver in ("v3",):
            tmp = DveOpSpec(name=name, opcode=row, uops=lower(spec, ver=ver),
                            rd1_en=False)
            shas[ver] = tmp.sha(ver)
        op = DveOp(name, spec, subdim=False, uops_sha=shas)
        OPS.append(op)
        CUSTOM_DVE_SPECS[name] = spec
        return op

    w = Src0 * C0
    base_body = C1 + w * (C1 + w * C2)

    def ref_base(in0, in1, s0, s1, imm2):
        ww = in0.astype(np.float32) * np.float32(s0)
        return (np.float32(s1) + ww * (np.float32(s1) + ww * np.float32(imm2))
                ).astype(np.float32)

    x = Src0
    for _ in range(8):
        x = sq(x)

    def ref_sq8(in0, in1, s0, s1, imm2):
        v = in0.astype(np.float32)
        for _ in range(8):
            v = (v * v).astype(np.float32)
        return v

    return make("EXP_BASE_XK", base_body, ref_base), make("EXP_SQ8_XK", x, ref_sq8)


def _build():
    import concourse.bacc as bacc
    import concourse.mybir as mybir
    import concourse.tile as tile

    EXP_BASE, EXP_SQ8 = _register_dve_exp()

    fp32 = mybir.dt.float32
    fp16 = mybir.dt.float16
    bf16 = mybir.dt.bfloat16

    nc = bacc.Bacc("TRN2", debug=False)
    xT = nc.dram_tensor("xT", [H, T], bf16, kind="ExternalInput").ap()
    wqkv = nc.dram_tensor("wqkv", [H, W3], bf16, kind="ExternalInput").ap()
    bqk = nc.dram_tensor("bqk", [2 * FG], fp32, kind="ExternalInput").ap()
    wp = nc.dram_tensor("wp", [FG, H], bf16, kind="ExternalInput").ap()
    out = nc.dram_tensor("out", [T, H], fp16, kind="ExternalOutput").ap()

    with tile.TileContext(nc) as tc:
        _emit(nc, tc, mybir, EXP_BASE, EXP_SQ8, xT, wqkv, bqk, wp, out)
    nc.compile()
    return nc


def _emit(nc, tc, mybir, EXP_BASE, EXP_SQ8, xT, wqkv, bqk, wp, out):
    from contextlib import ExitStack

    fp32 = mybir.dt.float32
    fp16 = mybir.dt.float16
    bf16 = mybir.dt.bfloat16
    Exp = mybir.ActivationFunctionType.Exp
    Ident = mybir.ActivationFunctionType.Identity

    with ExitStack() as ctx:
        pool = lambda name, bufs=1, space="SBUF": ctx.enter_context(
            tc.tile_pool(name=name, bufs=bufs, space=space)
        )

        # ---- persistent SBUF tiles (fine-grained so deps stay fine) ----
        const = pool("const")
        bias0 = const.tile([P, 1], fp32)
        nc.vector.memset(bias0[:], 0.0)
        junk = const.tile([P, NQ], bf16)        # PE warmup operand
        nc.vector.memset(junk[:], 0.001)
        warm = const.tile([P, 8], fp32)
        nc.vector.memset(warm[:], 0.0)
        bqk_sb = const.tile([P, 4], fp32)       # bias col per m-chunk
        nc.sync.dma_start(bqk_sb[:], bqk.rearrange("(m p) -> p m", p=P))

        xtp = pool("xt")
        # per (hidden chunk, token half) so early readers don't wait all DMA
        xts = [[xtp.tile([P, T // 2], bf16, name=f"xt{k}_{h}") for h in range(2)] for k in range(KH)]
        wqp = pool("wq")
        # q/k and v weight columns in separate tiles so the critical q/k
        # prefix DMA isn't diluted by v columns
        wqks = [wqp.tile([P, 2 * FG], bf16, name=f"wqk{k}") for k in range(KH)]
        wvs = [wqp.tile([P, FG], bf16, name=f"wv{k}") for k in range(KH)]
        wpp = pool("wpp")
        wps = [wpp.tile([P, H], bf16, name=f"wp{k}") for k in range(2)]
        qkp = pool("qk")
        qks = [[qkp.tile([P, NQ], bf16, name=f"qk{m}_{j}") for j in range(NJ)] for m in range(4)]
        vp = pool("v")
        vts = [vp.tile([P, 4 * P], bf16, name=f"v{t}") for t in range(NI)]
        yp = pool("y")
        ys = [[yp.tile([P, NQ], bf16, name=f"y{k}_{j}") for j in range(NJ)] for k in range(2)]

        # ---- working pools ----
        epool = pool("e", bufs=4)
        scrp = pool("scr", bufs=2)      # DVE exp fp32 intermediate
        pvsp = pool("pvs", bufs=4)      # staged PV numerators
        rcpp = pool("rcp", bufs=4)      # 1/denom rows + broadcasts
        outp = pool("o", bufs=4)        # fp16 out staging
        ps = ctx.enter_context(tc.tile_pool(name="ps", bufs=2, space="PSUM"))

        # ---- warmups (off critical path): ACT exp table, PE p-state ----
        nc.scalar.activation(warm[:], warm[:], Exp, bias=bias0[:, 0:1])
        for t in range(NI):  # mark ones col + zero pad of v tiles
            v4 = vts[t].rearrange("p (h c) -> p h c", h=4, c=P)
            nc.vector.memset(v4[:, :, 64:66], 0.0)
            nc.vector.memset(v4[:, :, 64:65], 1.0)

        # ---- input DMAs, spread across idle engine queues (not ACT:
        # scalar-queue DMA issues would steal ~6us from the exp stream) ----
        dq = [nc.sync, nc.gpsimd]
        di = [0]

        def dma(dst, src):
            dq[di[0] % 2].dma_start(dst, src)
            di[0] += 1

        for k in range(KH):  # critical prefix: q/k weights + first token half
            dma(wqks[k][:], wqkv[k * P : (k + 1) * P, 0 : 2 * FG])
            dma(xts[k][0][:], xT[k * P : (k + 1) * P, 0 : 2 * NQ])
        for k in range(KH):  # v weights (needed by the lead proj_v calls)
            dma(wvs[k][:], wqkv[k * P : (k + 1) * P, 2 * FG :])
        for k in range(KH):
            dma(xts[k][1][:], xT[k * P : (k + 1) * P, 2 * NQ :])
        for kk in range(2):
            dma(wps[kk][:], wp[kk * P : (kk + 1) * P, :])

        # PE warmup: junk matmuls to ramp the p-state while DMA lands.
        # jw[0] rotates the junk psum tag; extra junk is interleaved into
        # the lead-in so the HAM never sees a >3.4us idle window there.
        jw = [0]

        def junk_mm(tag="fill"):
            pw = ps.tile([P, NQ], fp32, tag=tag, name=f"warm{jw[0]}")
            nc.tensor.matmul(pw[:], junk[:, 0:P], junk[:], start=True, stop=True)
            jw[0] += 1

        for w in range(16):
            junk_mm()

        # ---- building blocks ----
        def proj_qk(m, jpair, on_act=False, lead_junk=False, jjs=(0, 1)):
            """q/k projection for feature chunk m, j-blocks 2*jpair+jjs.
            Weights per (m,k) load once and serve the jj matmuls. Bias-add
            copies go to ACT only during the lead-in (idle then); fillers
            use DVE so they never wedge between exps in the ACT FIFO.
            lead_junk: pad each chunk with a junk matmul so per-chunk DMA
            waits never accumulate into a HAM idle window."""
            pj = {
                jj: ps.tile([P, NQ], fp32, tag="fill", name=f"qk{m}_{jpair}_{jj}")
                for jj in jjs
            }
            for k in range(KH):
                for jj in jjs:
                    nc.tensor.matmul(
                        pj[jj][:],
                        wqks[k][:, m * P : (m + 1) * P],
                        xts[k][jpair][:, jj * NQ : (jj + 1) * NQ],
                        start=(k == 0),
                        stop=(k == KH - 1),
                    )
                if lead_junk:
                    junk_mm(tag="pvd")
            for jj in jjs:
                j = 2 * jpair + jj
                if on_act:
                    nc.scalar.activation(
                        qks[m][j][:], pj[jj][:], Ident, bias=bqk_sb[:, m : m + 1]
                    )
                else:
                    nc.vector.tensor_scalar_add(
                        qks[m][j][:], pj[jj][:], bqk_sb[:, m : m + 1]
                    )

        def proj_v(t, on_act=False):
            """v for token chunk t -> vts[t] cols [64h,64h+64) per head.
            The psum->SBUF cast rides on ACT in the PE-bound front phase."""
            pv = ps.tile([P, NQ], fp32, tag="fill", name=f"v{t}")
            th, tl = divmod(t, 8)
            for k in range(KH):
                nc.tensor.matmul(
                    pv[:, 0:FG],
                    xts[k][th][:, tl * P : (tl + 1) * P],
                    wvs[k][:],
                    start=(k == 0),
                    stop=(k == KH - 1),
                )
            v4 = vts[t].rearrange("p (h c) -> p h c", h=4, c=P)
            src = pv[:, 0:FG].rearrange("p (h c) -> p h c", h=4, c=64)
            if on_act:
                nc.scalar.copy(v4[:, :, 0:64], src)
            else:
                nc.vector.tensor_copy(v4[:, :, 0:64], src)

        def cproj(j, mql, tail=False, ptag="fill"):
            """c_proj for token rows j*512+128*mql. In the post-exp tail the
            psum->fp16 copies split across ACT (idle by then) and DVE, and
            ptag alternates over the attention psum tags (free by then) so
            consecutive cprojs don't serialize on fill-slot release."""
            pcs = [
                ps.tile([P, NQ], fp32, tag=ptag, name=f"c{j}_{mql}_{n}")
                for n in range(2)
            ]
            for kk in range(2):
                for n in range(2):
                    nc.tensor.matmul(
                        pcs[n][:],
                        ys[kk][j][:, mql * P : (mql + 1) * P],
                        wps[kk][:, n * NQ : (n + 1) * NQ],
                        start=(kk == 0),
                        stop=(kk == 1),
                    )
            r0 = j * NQ + mql * P
            for n in range(2):
                ot = outp.tile([P, NQ], fp16, tag="o", name=f"ot{j}_{mql}_{n}")
                if tail and n == 0:
                    nc.scalar.copy(ot[:], pcs[n][:])
                else:
                    nc.vector.tensor_copy(ot[:], pcs[n][:])
                nc.sync.dma_start(out[r0 : r0 + P, n * NQ : (n + 1) * NQ], ot[:])

        # ---- filler schedule: callables interleaved into group streams ----
        # Group order runs all pair-0 heads first, then pair-1, so the
        # pair-1 projections spread across the pair-0 groups and the cproj
        # work spreads across pair-1 groups: every group carries ~3.5us of
        # filler PE work instead of the front two carrying all of it.
        def F(fn, *a, **kw):
            return lambda: fn(*a, **kw)

        GROUP_ORDER = [(0, 0), (1, 0), (2, 0), (3, 0),
                       (0, 1), (1, 1), (2, 1), (3, 1)]
        # cproj fillers start at step 8 (foffset) so the producing group's
        # normalize chain has long drained before the cproj LDWEIGHTS waits
        # on ys -- a step-0 cproj stalls the in-order PE ~2.5us.
        fillers = {
            (0, 0): ([F(proj_qk, 2, 1)]
                     + [F(proj_v, t, on_act=True) for t in range(8, 16)], 0),
            (1, 0): ([F(proj_qk, 0, 1)], 0),
            (2, 0): ([F(proj_qk, 3, 0)], 0),
            (3, 0): ([F(proj_qk, 3, 1), F(proj_qk, 1, 0, jjs=(0,))], 0),
            (0, 1): ([F(proj_qk, 1, 0, jjs=(1,)), F(proj_qk, 1, 1)], 0),
            (1, 1): ([F(cproj, 0, q) for q in range(4)], 8),
            (2, 1): ([F(cproj, 1, q) for q in range(4)], 8),
            (3, 1): ([F(cproj, 2, q) for q in range(4)], 8),
        }
        # i-steps whose exp runs on DVE (per group): used in ACT-bound groups
        DVE_EXP = {g: DVE_EXP_I.get(g, ()) for g in GROUP_ORDER}

        def emit_s(j, p, i):
            sp = ps.tile([P, 2 * NQ], fp32, tag="s", name=f"s{j}{p}{i}")
            jb, ib = divmod(i, 4)
            for hh in range(2):
                nc.tensor.matmul(
                    sp[:, hh * NQ : (hh + 1) * NQ],
                    qks[2 + p][jb][64 * hh : 64 * hh + 64, ib * P : (ib + 1) * P],
                    qks[p][j][64 * hh : 64 * hh + 64, :],
                    start=True,
                    stop=True,
                    tile_position=(64 * hh, 0),
                )
            return sp

        def attend(j, p, pre, fo=0, first_sps=None, next_group=None, last=False):
            """16 i-steps for (j, pair p); pre = fillers list starting at
            step fo. S(0) and S(1) of the NEXT group are emitted during the
            last two steps so the exp stream never drains across group
            boundaries (the S pipeline stays two tiles deep)."""
            pvd = [
                ps.tile([P, NQ], fp32, tag="pvd", name=f"pvd{j}{p}{hh}")
                for hh in range(2)
            ]

            def emit_exp(i, sp):
                e = epool.tile([P, 2 * NQ], bf16, tag="e", name=f"e{j}{p}{i}")
                if i in DVE_EXP[(j, p)]:
                    scr = scrp.tile([P, 2 * NQ], fp32, tag="scr", name=f"sc{j}{p}{i}")
                    nc.vector._custom_dve(
                        EXP_BASE, out=scr[:], in0=sp[:],
                        s0=1.0 / 256, s1=1.0, imm2=0.5,
                    )
                    nc.vector._custom_dve(EXP_SQ8, out=e[:], in0=scr[:])
                else:
                    nc.scalar.activation(e[:], sp[:], Exp, bias=bias0[:, 0:1])
                return e

            # software-pipelined emission: the S pipeline runs two tiles
            # ahead of the exps (S(i+2) emitted at step i), and PV(i) is
            # DEFERRED one step (three for DVE exps, whose 2-op latency is
            # ~2.5us, and for i=0, which would otherwise block the in-order
            # PE on the previous group's pvd-freeing pvs copies).
            nf = len(pre)
            fi = 0
            sps = list(first_sps) if first_sps is not None else [
                emit_s(j, p, 0), emit_s(j, p, 1)]
            next_sps = []
            pend = []  # (i, e_tile, due_step), ordered by i

            def emit_pv(i0, e0):
                for hh in range(2):
                    nc.tensor.matmul(
                        pvd[hh][:],
                        vts[i0][:, (2 * p + hh) * P : (2 * p + hh + 1) * P],
                        e0[:, hh * NQ : (hh + 1) * NQ],
                        start=(i0 == 0),
                        stop=(i0 == NI - 1),
                    )

            def flush(cur):
                while pend and pend[0][2] <= cur:
                    i0, e0, _ = pend.pop(0)
                    emit_pv(i0, e0)

            for i in range(NI):
                e = emit_exp(i, sps[i])
                if i + 2 < NI:
                    sps.append(emit_s(j, p, i + 2))
                elif next_group is not None:
                    next_sps.append(emit_s(next_group[0], next_group[1], i + 2 - NI))
                due = i + 1
                if i == 0 or i in DVE_EXP[(j, p)]:
                    due = i + 3
                pend.append((i, e, due))
                flush(i)
                while fi < nf and i >= fo and fi * (NI - fo) <= (i - fo) * nf:
                    pre[fi]()
                    fi += 1
            while fi < nf:
                pre[fi]()
                fi += 1
            flush(NI + 3)
            # normalize: 1/denom from psum row 64, broadcast, multiply.
            # pvd psum is needed by the NEXT group's PVs, so the pvd reads
            # (dh + pvs copies) all run before the recip/broadcast/mul
            # chain. In the tail (last) the copies ride the then-idle ACT.
            dhs, pvss = [], []
            cp = nc.scalar.copy if last else nc.vector.tensor_copy
            for hh in range(2):
                dh = rcpp.tile([1, NQ], fp32, tag="d", name=f"dh{j}{p}{hh}")
                # custom DVE ops don't shift partitions: stage row 64 to p0
                cp(dh[:], pvd[hh][64:65, :])
                pvs = pvsp.tile([64, NQ], fp32, tag="pvs", name=f"pvs{j}{p}{hh}")
                cp(pvs[:], pvd[hh][0:64, :])
                dhs.append(dh)
                pvss.append(pvs)
            for hh in range(2):
                rh = rcpp.tile([1, NQ], fp32, tag="r", name=f"rh{j}{p}{hh}")
                nc.vector.reciprocal_approx_fast(out=rh[:], in_=dhs[hh][:])
                bc = rcpp.tile([64, NQ], fp32, tag="b", name=f"bc{j}{p}{hh}")
                nc.gpsimd.partition_broadcast(bc[:], rh[0:1, :])
                nc.vector.tensor_mul(
                    ys[p][j][64 * hh : 64 * hh + 64, :], pvss[hh][:], bc[:]
                )
            return next_sps

        # ---- main schedule ----
        proj_qk(2, 0, on_act=True, lead_junk=True)  # k pair0 j01
        proj_qk(0, 0, on_act=True, lead_junk=True)  # q pair0 j01
        for t in range(8):           # first-half v chunks ride the DMA shadow
            proj_v(t, on_act=True)
            if t < 4:
                junk_mm(tag="pvd")
        sps = None
        for gi, (j, p) in enumerate(GROUP_ORDER):
            nxt = GROUP_ORDER[gi + 1] if gi + 1 < len(GROUP_ORDER) else None
            pre, fo = fillers[(j, p)]
            sps = attend(j, p, pre, fo=fo, first_sps=sps, next_group=nxt,
                         last=(nxt is None))
        # tail: all four cproj(3,*) accumulate their pair-0 half right away
        # (ys[0][3] has been ready since group (3,0)); the pair-1 half and
        # the copies/DMAs follow once the last normalize lands. All 8 psum
        # banks are free here, so the four cprojs live concurrently across
        # the s/fill/pvd tags ("pvd" last: its slots free only after the
        # normalize's pvd reads). Junk matmuls bridge the remaining wait.
        for w in range(6):
            junk_mm()
        tails = {}
        TAIL_ORDER = [(0, "s"), (1, "s"), (3, "fill"), (2, "pvd")]
        for q, tag in TAIL_ORDER:
            if tag == "s":
                pc = ps.tile([P, 2 * NQ], fp32, tag="s", name=f"ct{q}")
                pcs = [pc[:, 0:NQ], pc[:, NQ : 2 * NQ]]
            else:
                pcs = [
                    ps.tile([P, NQ], fp32, tag=tag, name=f"ct{q}_{n}")[:]
                    for n in range(2)
                ]
            for n in range(2):
                nc.tensor.matmul(
                    pcs[n],
                    ys[0][3][:, q * P : (q + 1) * P],
                    wps[0][:, n * NQ : (n + 1) * NQ],
                    start=True,
                    stop=False,
                )
            tails[q] = pcs
        for q, _ in TAIL_ORDER:
            pcs = tails[q]
            for n in range(2):
                nc.tensor.matmul(
                    pcs[n],
                    ys[1][3][:, q * P : (q + 1) * P],
                    wps[1][:, n * NQ : (n + 1) * NQ],
                    start=False,
                    stop=True,
                )
            r0 = 3 * NQ + q * P
            for n in range(2):
                ot = outp.tile([P, NQ], fp16, tag="o", name=f"ott{q}_{n}")
                if n == 0:
                    nc.scalar.copy(ot[:], pcs[n])
                else:
                    nc.vector.tensor_copy(ot[:], pcs[n])
                nc.sync.dma_start(out[r0 : r0 + P, n * NQ : (n + 1) * NQ], ot[:])


def _get_nc():
    if "nc" not in _CACHE:
        _CACHE["nc"] = _build()
    return _CACHE["nc"]


def _make_in_maps(x, W_attn, b_attn, W_proj):
    import ml_dtypes

    bf = ml_dtypes.bfloat16
    x = np.asarray(x, np.float32)
    W_attn = np.asarray(W_attn, np.float32)
    b_attn = np.asarray(b_attn, np.float32)
    scale = 1.0 / np.sqrt(np.float32(HD))
    W_proj = np.asarray(W_proj, np.float32)
    in_maps = []
    for c in range(NCORES):
        b, g = divmod(c, 4)
        sl = slice(FG * g, FG * (g + 1))
        wq = W_attn[:, sl] * scale
        wk = W_attn[:, H:][:, sl]
        wv = W_attn[:, 2 * H :][:, sl]
        in_maps.append(
            {
                "xT": np.ascontiguousarray(x[b].T).astype(bf),
                "wqkv": np.ascontiguousarray(
                    np.concatenate([wq, wk, wv], axis=1)
                ).astype(bf),
                "bqk": np.concatenate(
                    [b_attn[sl] * scale, b_attn[H:][sl]]
                ).astype(np.float32),
                "wp": np.ascontiguousarray(W_proj[sl, :]).astype(bf),
            }
        )
    return in_maps


def _gather(results, b_attn, W_proj, b_proj):
    b_attn = np.asarray(b_attn, np.float64)
    W_proj = np.asarray(W_proj, np.float64)
    b_proj = np.asarray(b_proj, np.float64)
    # v-bias commutes through softmax: y = sum_k p_k (v_k + bv) = y0 + bv
    host_bias = (b_attn[2 * H :] @ W_proj + b_proj).astype(np.float32)
    y = np.empty((B, T, H), np.float32)
    for b in range(B):
        acc = results[4 * b]["out"].astype(np.float32)
        for g in range(1, 4):
            acc = acc + results[4 * b + g]["out"].astype(np.float32)
        y[b] = acc + host_bias[None, :]
    return y


def run(x, W_attn, b_attn, W_proj, b_proj, trace=False):
    from concourse.bass_utils import run_bass_kernel_spmd

    nc = _get_nc()
    in_maps = _make_in_maps(x, W_attn, b_attn, W_proj)
    res = run_bass_kernel_spmd(nc, in_maps, list(range(NCORES)), trace=trace)
    return _gather(res.results, b_attn, W_proj, b_proj), res


def kernel(x, W_attn, b_attn, W_proj, b_proj):
    y, _ = run(x, W_attn, b_attn, W_proj, b_proj, trace=False)
    return y

